# revision 18
# baseline (speedup 1.0000x reference)
# Trainium2 Bass kernel for a binarized 2-block MLP (BNN):
#   h1 = sign(BN1(x @ sign(w1).T + b1)); h2 = sign(BN2(h1 @ sign(w2).T + b2))
#   out = log_softmax(h2 @ sign(w5).T + b5)
#
# Strategy: pure data parallel over 8 NeuronCores (batch sharded, weights
# replicated). Host-side prep:
#   * x is split into fp16 hi/lo parts (x == xh + xl/2048 to ~2^-22 in fp32).
#     Both stream through the PE at full (1 col/cycle) rate vs fp32's 4
#     passes; with +-1 binary weights every product is exact in the PE's
#     FP22 pipe, so the result matches a true fp32 matmul to accumulation
#     order.
#   * BN is folded into per-feature thresholds applied by the DVE's is_ge.
#   * b5 is folded in on the vector engine: sum(exp(mm+b5)) via a
#     multiply-reduce against exp(b5), and the final subtract adds b5 in
#     the same fused op.
#
# The kernel sits at the HBM roofline: per core 32MB of x streams at
# ~380-400GB/s (~81-84us). The PE's work (~4.6-5.1us/chunk vs the ~5.1us
# chunk DMA period) fits just under the stream, so the schedule keeps the
# PE STRICTLY DMA-PACED: never starting late, never idling >~1us (the HAM
# activity monitor halves the clock for >=3.4us after a >~1us idle).
# Hard-won scheduling facts baked in:
#   * Each chunk's x rides as TWO whole-chunk 1MB DMAs: hi on the sync
#     (SP) HWDGE ring, lo on the scalar (ACT) ring. Whole chunks (8KB
#     contiguous per partition) are mandatory: splitting each into 4KB
#     k-half pieces measured 5% slower on the wire AND doubled the DMA
#     count per engine queue, whose semaphore-reuse waits then blocked the
#     exp/ln ACTIVATEs queued behind them (ACT late -> DVE late -> PE
#     fc2/fc5 stalls -> HAM throttle).
#   * The two rings MUST carry equal bytes. v1 put w1h+w1l and all six
#     const DMAs on the sync ring, which put the hi lane ~4us behind the
#     lo lane for the entire run (chunk completion = max(hi, lo)): every
#     chunk landed ~4us late and the PE never caught up, leaving a ~20us
#     serial tail after the stream. Now w1h rides sync, w1l rides scalar,
#     and the consts are split 3/3 between rings, slotted between chunk
#     0/1 DMAs (each const DMA costs ~0.6-0.9us of in-order HWDGE
#     descriptor-gen on its engine, so they must not precede chunk 0).
#   * The last chunk is stored in DRAM as two 256-col half-chunks (host
#     repack) so its fc1 can start when the first half lands, 2.6us before
#     the stream ends; the halves pipeline through sign1->fc2->sign2->fc5->
#     softmax, with stage_b(14) slotted into fc1(15b)'s PE window, to keep
#     the post-stream tail to ~4us of cross-engine chain.
#   * The Tile scheduler emits each engine's instruction order from a
#     timed simulation whose DMA model underestimates HBM rate; the
#     two-ring split doubles the modeled rate so the emitted order keeps
#     the software pipeline (single-ring emitted a serialized
#     fc1->sign1->fc2 chain).
#   * All inter-stage PSUM pools are double-buffered; 8 PSUM banks exactly.
#   * fc1 weights lead the rings pre-swizzled+contiguous (an on-device
#     rearrange was 1024x100B descriptors, landing 8us late).
#   * PE idle >~1us re-throttles the HAM clock gate to half rate for
#     >=3.4us. Keep-warm filler matmuls bridge the gaps: a dependency-free
#     warmup burst sized to end when chunk 0 lands, then a tapering count
#     over chunks 1-4 (the pipeline ramp is the only phase where the PE
#     outpaces the stream by >1us), anchored on the PREVIOUS chunk's hi
#     tile and emitted before fc1(c) so they fill the pre-chunk wait.
#     Fillers write an unread PSUM bank, double-buffered so their WAW
#     waits are always two groups old.
#   * Output is never transposed: softmax results accumulate in one
#     SBUF-resident [128, nch, 4, O] tile and ship as one 14-chunk DMA
#     overlapped with the tail compute plus two tiny end pieces, so no
#     store traffic competes with the x stream.

import os
import sys

import numpy as np

for _p in ("/opt/trn_rl_repo", "/root/.axon_site/_ro/trn_rl_repo"):
    if os.path.isdir(_p) and _p not in sys.path:
        sys.path.insert(0, _p)

import concourse.bass as bass
import concourse.mybir as mybir
import concourse.tile as tile
from concourse import bacc
from concourse.masks import make_identity

N_CORES = 8
B, D, H1, H2, O = 65536, 1024, 50, 20, 10
BPC = B // N_CORES  # batch rows per core
CH = 512            # batch chunk (one PSUM bank of fp32)
KS = D // 128       # contraction slices
EPS = 1e-4
LO = 2048.0         # lo-part scale (2**11)
JB = CH // 128      # fc5 j-blocks per chunk
HW = CH // 2        # last-chunk column half

NBUF = 10           # x chunk buffers in flight (absorbs PE lag jitter so the
                    # DMA stream never stalls on buffer recycling)
WARMUP_FILL = 27    # ident fillers before chunk 0 lands (~5.7-6.8us)
FILL_AT = {1: 14, 2: 10, 3: 7, 4: 4}   # ramp-phase keep-warm fillers (256-col)

F16 = mybir.dt.float16
F32 = mybir.dt.float32
AF = mybir.ActivationFunctionType
AX = mybir.AxisListType
OP = mybir.AluOpType


def build_bass(bpc: int = BPC) -> bass.Bass:
    nch = bpc // CH
    nfull = nch - 1      # full 512-row chunks; the last is two 256-col halves
    nc = bacc.Bacc("TRN2", target_bir_lowering=False)

    # All activations used here (Exp, Ln) live together in the
    # "natural_log_exp_and_others" ACT table set, but the default chooser
    # first-matches Exp->exp_and_others and Ln->natural_log, reloading
    # tables twice per chunk (~2.7us each). Restrict the chooser to the
    # combined set (other entries emptied so indices stay aligned with
    # act_info.json) -> exactly one table load for the whole kernel.
    def _act_table_loads_combined_set_only(self=nc):
        import bass_rust as _br

        from concourse.hw_specs import get_activation_tables

        has_act = any(
            isinstance(i, mybir.InstActivation)
            for blk in self.main_func.blocks
            for i in blk.instructions
        )
        if not has_act:
            return
        tabs = get_activation_tables(self.m.arch)
        tables = [
            (name, fns if name == "natural_log_exp_and_others" else set())
            for name, fns in tabs.items()
        ]
        _br.insert_act_table_loads(self, tables)

    nc.insert_act_table_loads = _act_table_loads_combined_set_only

    # x arrives pre-swizzled with hi/lo interleaved per chunk:
    #   x2[p, c, s, k, n] = (xh if s==0 else xl).T[k*128+p, c*CH+n]
    # so each chunk's per-partition DMA slice is one contiguous 8KB run.
    x2 = nc.declare_dram_parameter("x2", [128, nfull, 2, KS, CH], F16,
                                   isOutput=False)
    # last chunk as two 256-col halves, hi/lo (4KB runs):
    #   x2t[p, i, s, k, n] = (xh if s==0 else xl).T[k*128+p, nfull*CH+i*HW+n]
    x2t = nc.declare_dram_parameter("x2t", [128, 2, 2, KS, HW], F16,
                                    isOutput=False)
    # w1 halves pre-swizzled to [128, KS*H1]: one 800B contiguous run per
    # partition.
    w1h = nc.declare_dram_parameter("w1h", [128, KS * H1], F16, isOutput=False)
    w1l = nc.declare_dram_parameter("w1l", [128, KS * H1], F16, isOutput=False)
    w2t = nc.declare_dram_parameter("w2t", [H1, H2], F16, isOutput=False)
    w5t = nc.declare_dram_parameter("w5t", [H2, O], F16, isOutput=False)
    cs1 = nc.declare_dram_parameter("cs1", [H1, 2], F32, isOutput=False)
    cs2 = nc.declare_dram_parameter("cs2", [H2, 2], F32, isOutput=False)
    b5j = nc.declare_dram_parameter("b5j", [1, JB, O], F32, isOutput=False)
    # Output, stored untransposed, batch-on-partition:
    #   y[p, c, j, o] = out[c*512 + j*128 + p, o]
    y = nc.declare_dram_parameter("y", [128, nch, JB, O], F32, isOutput=True)

    with tile.TileContext(nc) as tc:
        from contextlib import ExitStack

        with ExitStack() as ctx:
            singles = ctx.enter_context(tc.tile_pool(name="singles", bufs=1))
            xpool = ctx.enter_context(tc.tile_pool(name="xpool", bufs=NBUF))
            tpool = ctx.enter_context(tc.tile_pool(name="tpool", bufs=1))
            mids = ctx.enter_context(tc.tile_pool(name="mids", bufs=3))
            p1pool = ctx.enter_context(tc.tile_pool(name="p1", bufs=2, space="PSUM"))
            p2pool = ctx.enter_context(tc.tile_pool(name="p2", bufs=2, space="PSUM"))
            p5pool = ctx.enter_context(tc.tile_pool(name="p5", bufs=2, space="PSUM"))
            pFpool = ctx.enter_context(tc.tile_pool(name="pF", bufs=2, space="PSUM"))

            # Identity first: the gpsimd engine builds it in ~1us so the
            # warmup fillers can start right after the preamble.
            ident = singles.tile([128, 128], F32)
            make_identity(nc, ident)
            identb = singles.tile([128, 128], F32)
            make_identity(nc, identb)

            xts = []

            def issue_x(c):
                # ONE 2MB DMA per chunk (hi+lo adjacent in DRAM: one 16KB
                # contiguous run per partition), ALL on the sync (SP) HWDGE
                # ring. The scalar (ACT) ring carries NO x traffic: a DMA
                # desc-gen instruction blocks its engine's queue while it
                # waits (buffer-recycle WAR + semaphore-reuse wire pacing),
                # and with x-lo DMAs on the scalar engine those waits
                # queue-blocked the exp/ln ACTIVATEs behind them -> DVE
                # reduce late -> DVE in-order queue stuck -> sign1 late ->
                # PE fc2/fc5 stalls and a 3us PSUM-WAR stall at the drain.
                # The 16 SDMA engines drain a single HWDGE ring at the full
                # ~400GB/s, so one ring loses no wire rate. (gpsimd SWDGE
                # measured ~25% slower on the wire -- not an option.)
                xt = xpool.tile([128, 2, KS, CH], F16, tag="x", name="x_t")
                nc.sync.dma_start(out=xt, in_=x2[:, c])
                xts.append(xt)

            xtt = []

            def issue_xt():
                for i in range(2):
                    tt = tpool.tile([128, 2, KS, HW], F16, tag=f"t{i}", name=f"t{i}")
                    nc.sync.dma_start(out=tt, in_=x2t[:, i])
                    xtt.append(tt)

            # --- DMA preamble. fc1 weights lead the sync ring (fc1(0)
            # cannot start without them), then the x chunks back to back.
            # The consts ride the otherwise-empty scalar ring concurrently.
            w1h_sb = singles.tile([128, KS, H1], F16)
            nc.sync.dma_start(out=w1h_sb, in_=w1h[:, :])
            w1l_sb = singles.tile([128, KS, H1], F16)
            nc.sync.dma_start(out=w1l_sb, in_=w1l[:, :])
            issue_x(0)
            w2_sb = singles.tile([H1, H2], F16)
            nc.scalar.dma_start(out=w2_sb, in_=w2t[:, :])
            cs1_sb = singles.tile([H1, 2], F32)
            nc.scalar.dma_start(out=cs1_sb, in_=cs1[:, :])
            w5_sb = singles.tile([H2, O], F16)
            nc.scalar.dma_start(out=w5_sb, in_=w5t[:, :])
            cs2_sb = singles.tile([H2, 2], F32)
            nc.scalar.dma_start(out=cs2_sb, in_=cs2[:, :])
            b5j_sb = singles.tile([1, JB, O], F32)
            nc.scalar.dma_start(out=b5j_sb, in_=b5j[:, :])
            ones_sb = singles.tile([1, 128], F32)
            nc.gpsimd.memset(ones_sb, 1.0)
            out_all = singles.tile([128, nch, JB, O], F32)
            for c in range(1, min(NBUF, nfull)):
                issue_x(c)

            def fillers(n, xt=None):
                # Keep-warm matmuls: pad PE idle so the HAM activity monitor
                # never re-throttles the clock. One accumulation group -> no
                # per-instruction WAW semaphore stalls; output never read.
                # Warmup form (xt=None): fp32 ident@ident, dependency-free
                # so it runs during the pre-chunk-0 window. Anchored form
                # (256-col, ~0.15us each): reads an already-landed x tile,
                # so it executes immediately when the PE reaches it --
                # emitted just before fc1(c), it fills the pre-chunk data
                # wait without delaying anything.
                if n <= 0:
                    return
                fp = pFpool.tile([128, CH], F32, tag="f", name="fill")
                if xt is None:
                    for i in range(n):
                        nc.tensor.matmul(fp[:, 0:128],
                                         lhsT=(ident if i % 2 == 0 else identb),
                                         rhs=ident,
                                         start=(i == 0), stop=(i == n - 1))
                else:
                    for i in range(n):
                        nc.tensor.matmul(fp[:, 0:256], lhsT=xt[:, 0, 0, 0:128],
                                         rhs=xt[:, 0, 0, 0:256],
                                         start=(i == 0), stop=(i == n - 1))

            def fc1(c):
                xt = xts[c]
                ps1 = p1pool.tile([H1, CH], F32, tag="ps1", name="ps1")
                for k in range(KS):
                    nc.tensor.matmul(ps1, lhsT=w1h_sb[:, k, :], rhs=xt[:, 0, k, :],
                                     start=(k == 0), stop=False)
                for k in range(KS):
                    nc.tensor.matmul(ps1, lhsT=w1l_sb[:, k, :], rhs=xt[:, 1, k, :],
                                     start=False, stop=(k == KS - 1))
                return ps1

            def fc1_tail(i):
                tt = xtt[i]
                ps1 = p1pool.tile([H1, HW], F32, tag="ps1", name="ps1")
                for k in range(KS):
                    nc.tensor.matmul(ps1, lhsT=w1h_sb[:, k, :], rhs=tt[:, 0, k, :],
                                     start=(k == 0), stop=False)
                for k in range(KS):
                    nc.tensor.matmul(ps1, lhsT=w1l_sb[:, k, :], rhs=tt[:, 1, k, :],
                                     start=False, stop=(k == KS - 1))
                return ps1

            def stage_a(ps1, w=CH):
                """sign1 -> fc2 -> sign2 for one chunk; returns (y1, y2).
                The signs run on the vector engine as {0,1} comparisons
                (weights/thresholds pre-folded on host) -- the ACT engine
                was the per-chunk straggler (lo descriptor-gen + exp/ln),
                and late sign1 stalled fc2 on the PE every chunk. GPSIMD
                cannot touch PSUM, so the DVE is the only other option."""
                y1 = mids.tile([H1, w], F16, tag="y1", name="y1")
                nc.vector.tensor_scalar(y1, ps1, cs1_sb[:, 0:1], None,
                                        OP.is_ge)
                ps2 = p2pool.tile([H2, w], F32, tag="ps2", name="ps2")
                nc.tensor.matmul(ps2, lhsT=w2_sb, rhs=y1, start=True, stop=True)
                y2 = mids.tile([H2, w], F16, tag="y2", name="y2", bufs=4)
                nc.vector.tensor_scalar(y2, ps2, cs2_sb[:, 0:1], None,
                                        OP.is_ge)
                return y1, y2

            def stage_b(c, y2, jlo=0, njb=JB):
                """fc5 -> log_softmax for one chunk (njb j-blocks of 128);
                writes out_all[:, c, jlo:jlo+njb, :]. b5 (minus the sign
                colsum correction) rides INTO the fc5 PSUM accumulation as
                a rank-1 ones-x-b5row matmul opening the group (~0.1us PE)
                -- this removes the DVE multiply against exp(b5) and one
                cross-engine hop from the per-chunk softmax chain, keeping
                the DVE (the drain-phase straggler) to sign/reduce/sub."""
                ps5 = p5pool.tile([128, njb, O], F32, tag="ps5", name="ps5")
                nc.tensor.matmul(ps5, lhsT=ones_sb[0:1, :],
                                 rhs=b5j_sb[0:1, jlo:jlo + njb, :],
                                 start=True, stop=False)
                for j in range(njb):
                    nc.tensor.matmul(ps5[:, j, :], lhsT=y2[:, j * 128:(j + 1) * 128],
                                     rhs=w5_sb, start=False, stop=True)

                # log_softmax along free dim (|logits| <= 21 so no
                # max-subtraction is needed)
                e = mids.tile([128, njb, O], F32, tag="e", name="e")
                nc.scalar.activation(e, ps5, AF.Exp)
                s = mids.tile([128, njb], F32, tag="s", name="s")
                nc.vector.reduce_sum(s, e, axis=AX.X)
                lse = mids.tile([128, njb], F32, tag="lse", name="lse")
                nc.scalar.activation(lse, s, AF.Ln)
                for j in range(njb):
                    nc.vector.tensor_scalar(
                        out_all[:, c, jlo + j, :], ps5[:, j, :],
                        lse[:, j:j + 1], None, OP.subtract)

            # Software pipeline, one chunk per iteration:
            #   [ramp fillers] fc1(c) | stage_a(c-1) | stage_b(c-2) |
            #   issue chunk c+NBUF. Every PE instruction's inputs are >=1
            #   iteration old when the PE's in-order queue reaches it, so
            #   the PE only ever waits for the x DMAs -- and the fillers
            #   bridge those waits during the ramp.
            fillers(WARMUP_FILL)
            ps1s = {}
            y2s = {}
            for c in range(nfull):
                if c >= 1:
                    fillers(FILL_AT.get(c, 0), xt=xts[c - 1])
                ps1s[c] = fc1(c)
                if c >= 1:
                    _, y2s[c - 1] = stage_a(ps1s.pop(c - 1))
                if c >= 2:
                    stage_b(c - 2, y2s.pop(c - 2))
                if c + NBUF < nfull:
                    issue_x(c + NBUF)
                elif c + NBUF == nfull:
                    issue_xt()

            # Drain: the two last-chunk halves pipeline through the chain;
            # stage_b(14) is slotted into fc1(15b)'s PE window so its
            # DVE/ACT chain overlaps the last fc1, and the final two
            # half-chunk softmax chains are all that trails the stream.
            ps1a = fc1_tail(0)
            _, y2_14 = stage_a(ps1s.pop(nfull - 1))
            stage_b(nfull - 2, y2s.pop(nfull - 2))
            # bulk of the output ships now, overlapped with the tail
            # compute, on the by-then-idle sync ring
            nc.sync.dma_start(out=y[:, 0:nfull - 1], in_=out_all[:, 0:nfull - 1])
            _, y2a = stage_a(ps1a, w=HW)
            stage_b(nfull - 1, y2_14)
            nc.sync.dma_start(out=y[:, nfull - 1:nfull],
                              in_=out_all[:, nfull - 1:nfull])
            ps1b = fc1_tail(1)
            _, y2b = stage_a(ps1b, w=HW)
            stage_b(nfull, y2a, jlo=0, njb=JB // 2)
            stage_b(nfull, y2b, jlo=JB // 2, njb=JB // 2)
            # Keep the PE busy through the cross-engine drain chain: a PE
            # idle here trips the HAM half-clock right as the end-of-kernel
            # cleanup (barriers + semaphore resets, ~5us) starts, doubling
            # it. Anchored on the last tail tile so they can't be hoisted.
            fillers(20, xt=xtt[1])
            nc.scalar.dma_start(out=y[:, nfull:], in_=out_all[:, nfull:])

    nc.finalize()
    return nc


def _prep_inputs(x, w1, b1, g1, be1, m1, v1, w2, b2, g2, be2, m2, v2, w5, b5,
                 bpc: int = BPC, n_cores: int = N_CORES):
    f64 = np.float64
    w1s = np.where(w1 >= 0, 1.0, -1.0).astype(np.float32)
    w2s = np.where(w2 >= 0, 1.0, -1.0).astype(np.float32)
    w5s = np.where(w5 >= 0, 1.0, -1.0).astype(np.float32)

    def wswz(a):  # [D, H1] -> [128, KS*H1]: a.T[k*128+p, m] -> out[p, k*H1+m]
        return np.ascontiguousarray(
            a.reshape(KS, 128, H1).transpose(1, 0, 2).reshape(128, KS * H1))

    w1h = wswz(np.ascontiguousarray(w1s.T).astype(np.float16))
    w1l = wswz((np.ascontiguousarray(w1s.T) / LO).astype(np.float16))
    # sign activations run on the vector engine as b = (preact >= thr) in
    # {0,1}; the +-1 semantics fold into DOUBLED next-layer weights plus a
    # row/col-sum constant absorbed into the next threshold / bias
    # (y = 2b - 1  =>  y @ W = b @ 2W - sum(W)). Requires BN scale > 0,
    # which holds here (g = 1, v > 0).
    w2t = (2.0 * np.ascontiguousarray(w2s.T)).astype(np.float16)  # [H1, H2]
    w5t = (2.0 * np.ascontiguousarray(w5s.T)).astype(np.float16)  # [H2, O]
    rs2 = w2s.astype(f64).sum(axis=1)                             # [H2]
    cs5 = w5s.astype(f64).sum(axis=1)                             # [O]

    b5c = (b5.astype(f64) - cs5)
    b5jr = np.tile(b5c.astype(np.float32), 4).reshape(1, 4, O)

    s1 = (g1.astype(f64) / np.sqrt(v1.astype(f64) + EPS))
    t1 = s1 * (b1.astype(f64) - m1.astype(f64)) + be1.astype(f64)
    thr1 = -t1 / s1
    cs1 = np.stack([thr1, 0 * thr1], axis=1).astype(np.float32)   # [H1, 2]
    s2 = (g2.astype(f64) / np.sqrt(v2.astype(f64) + EPS))
    t2 = s2 * (b2.astype(f64) - m2.astype(f64)) + be2.astype(f64)
    thr2 = rs2 - t2 / s2
    cs2 = np.stack([thr2, 0 * thr2], axis=1).astype(np.float32)   # [H2, 2]

    x = np.asarray(x, dtype=np.float32)
    xh = x.astype(np.float16)
    xl = ((x - xh.astype(np.float32)) * LO).astype(np.float16)

    nfull = bpc // CH - 1

    def swizzle(a):  # [nfull*CH, D] -> [128, nfull, KS, CH]
        return np.ascontiguousarray(
            a.T.reshape(KS, 128, nfull, CH).transpose(1, 2, 0, 3))

    def swizzle_t(a):  # [CH, D] -> [128, 2, KS, HW]
        return np.ascontiguousarray(
            a.T.reshape(KS, 128, 2, HW).transpose(1, 2, 0, 3))

    in_maps = []
    for c in range(n_cores):
        rs = slice(c * bpc, c * bpc + nfull * CH)
        rt = slice(c * bpc + nfull * CH, (c + 1) * bpc)
        # x2[p, c, s, k, n]; x2t[p, i, s, k, n]
        x2 = np.ascontiguousarray(
            np.stack([swizzle(xh[rs]), swizzle(xl[rs])], axis=2))
        x2tt = np.ascontiguousarray(
            np.stack([swizzle_t(xh[rt]), swizzle_t(xl[rt])], axis=2))
        in_maps.append({
            "x2": x2, "x2t": x2tt,
            "w1h": w1h, "w1l": w1l, "w2t": w2t, "w5t": w5t,
            "cs1": cs1, "cs2": cs2, "b5j": b5jr,
        })
    return in_maps


def _decode_output(y_dev: np.ndarray, bpc: int) -> np.ndarray:
    # y_dev [128, nch, 4, O]: y_dev[p, c, j, o] = out[c*512 + j*128 + p, o]
    return np.ascontiguousarray(
        y_dev.transpose(1, 2, 0, 3).reshape(bpc, O))


_CACHED = {}


def kernel(**inputs) -> np.ndarray:
    from concourse.bass_utils import run_bass_kernel_spmd

    in_maps = _prep_inputs(**inputs)
    if "nc" not in _CACHED:
        _CACHED["nc"] = build_bass()
    nc = _CACHED["nc"]
    res = run_bass_kernel_spmd(nc, in_maps, list(range(N_CORES)))
    out = np.empty((B, O), dtype=np.float32)
    for c in range(N_CORES):
        out[c * BPC:(c + 1) * BPC] = _decode_output(res.results[c]["y"], BPC)
    return out


# revision 24
# speedup vs baseline: 1.2187x; 1.2187x over previous
# Trainium2 Bass kernel for a binarized 2-block MLP (BNN):
#   h1 = sign(BN1(x @ sign(w1).T + b1)); h2 = sign(BN2(h1 @ sign(w2).T + b2))
#   out = log_softmax(h2 @ sign(w5).T + b5)
#
# Strategy: pure data parallel over 8 NeuronCores (batch sharded, weights
# replicated). Host-side prep:
#   * x is split into fp16 hi/lo parts (x == xh + xl/2048 to ~2^-22 in fp32).
#     Both stream through the PE at full (1 col/cycle) rate vs fp32's 4
#     passes; with +-1 binary weights every product is exact in the PE's
#     FP22 pipe, so the result matches a true fp32 matmul to accumulation
#     order.
#   * BN is folded into per-feature thresholds applied by the DVE's is_ge.
#   * b5 is folded in on the vector engine: sum(exp(mm+b5)) via a
#     multiply-reduce against exp(b5), and the final subtract adds b5 in
#     the same fused op.
#
# The kernel sits at the HBM roofline: per core 32MB of x streams at
# ~380-400GB/s (~81-84us). The PE's work (~4.6-5.1us/chunk vs the ~5.1us
# chunk DMA period) fits just under the stream, so the schedule keeps the
# PE STRICTLY DMA-PACED: never starting late, never idling >~1us (the HAM
# activity monitor halves the clock for >=3.4us after a >~1us idle).
# Hard-won scheduling facts baked in:
#   * Each chunk's x rides as TWO whole-chunk 1MB DMAs: hi on the sync
#     (SP) HWDGE ring, lo on the scalar (ACT) ring. Whole chunks (8KB
#     contiguous per partition) are mandatory: splitting each into 4KB
#     k-half pieces measured 5% slower on the wire AND doubled the DMA
#     count per engine queue, whose semaphore-reuse waits then blocked the
#     exp/ln ACTIVATEs queued behind them (ACT late -> DVE late -> PE
#     fc2/fc5 stalls -> HAM throttle).
#   * The two rings MUST carry equal bytes. v1 put w1h+w1l and all six
#     const DMAs on the sync ring, which put the hi lane ~4us behind the
#     lo lane for the entire run (chunk completion = max(hi, lo)): every
#     chunk landed ~4us late and the PE never caught up, leaving a ~20us
#     serial tail after the stream. Now w1h rides sync, w1l rides scalar,
#     and the consts are split 3/3 between rings, slotted between chunk
#     0/1 DMAs (each const DMA costs ~0.6-0.9us of in-order HWDGE
#     descriptor-gen on its engine, so they must not precede chunk 0).
#   * The last chunk is stored in DRAM as two 256-col half-chunks (host
#     repack) so its fc1 can start when the first half lands, 2.6us before
#     the stream ends; the halves pipeline through sign1->fc2->sign2->fc5->
#     softmax, with stage_b(14) slotted into fc1(15b)'s PE window, to keep
#     the post-stream tail to ~4us of cross-engine chain.
#   * The Tile scheduler emits each engine's instruction order from a
#     timed simulation whose DMA model underestimates HBM rate; the
#     two-ring split doubles the modeled rate so the emitted order keeps
#     the software pipeline (single-ring emitted a serialized
#     fc1->sign1->fc2 chain).
#   * All inter-stage PSUM pools are double-buffered; 8 PSUM banks exactly.
#   * fc1 weights lead the rings pre-swizzled+contiguous (an on-device
#     rearrange was 1024x100B descriptors, landing 8us late).
#   * PE idle >~1us re-throttles the HAM clock gate to half rate for
#     >=3.4us. Keep-warm filler matmuls bridge the gaps: a dependency-free
#     warmup burst sized to end when chunk 0 lands, then a tapering count
#     over chunks 1-4 (the pipeline ramp is the only phase where the PE
#     outpaces the stream by >1us), anchored on the PREVIOUS chunk's hi
#     tile and emitted before fc1(c) so they fill the pre-chunk wait.
#     Fillers write an unread PSUM bank, double-buffered so their WAW
#     waits are always two groups old.
#   * Output is never transposed: softmax results accumulate in one
#     SBUF-resident [128, nch, 4, O] tile and ship as one 14-chunk DMA
#     overlapped with the tail compute plus two tiny end pieces, so no
#     store traffic competes with the x stream.

import os
import sys

import numpy as np

for _p in ("/opt/trn_rl_repo", "/root/.axon_site/_ro/trn_rl_repo"):
    if os.path.isdir(_p) and _p not in sys.path:
        sys.path.insert(0, _p)

import concourse.bass as bass
import concourse.mybir as mybir
import concourse.tile as tile
from concourse import bacc
from concourse.masks import make_identity

N_CORES = 8
B, D, H1, H2, O = 65536, 1024, 50, 20, 10
BPC = B // N_CORES  # batch rows per core
CH = 512            # batch chunk (one PSUM bank of fp32)
KS = D // 128       # contraction slices
EPS = 1e-4
LO = 2048.0         # lo-part scale (2**11)
JB = CH // 128      # fc5 j-blocks per chunk
HW = CH // 2        # last-chunk column half

NBUF = 10           # x chunk buffers in flight (absorbs PE lag jitter so the
                    # DMA stream never stalls on buffer recycling)
WARMUP_FILL = 27    # ident fillers before chunk 0 lands (~5.7-6.8us)
FILL_AT = {1: 14, 2: 10, 3: 7, 4: 4}   # ramp-phase keep-warm fillers (256-col)

F16 = mybir.dt.float16
F32 = mybir.dt.float32
AF = mybir.ActivationFunctionType
AX = mybir.AxisListType
OP = mybir.AluOpType


def build_bass(bpc: int = BPC) -> bass.Bass:
    nch = bpc // CH
    nfull = nch - 1      # full 512-row chunks; the last is two 256-col halves
    nc = bacc.Bacc("TRN2", target_bir_lowering=False)

    # All activations used here (Exp, Ln) live together in the
    # "natural_log_exp_and_others" ACT table set, but the default chooser
    # first-matches Exp->exp_and_others and Ln->natural_log, reloading
    # tables twice per chunk (~2.7us each). Restrict the chooser to the
    # combined set (other entries emptied so indices stay aligned with
    # act_info.json) -> exactly one table load for the whole kernel.
    def _act_table_loads_combined_set_only(self=nc):
        import bass_rust as _br

        from concourse.hw_specs import get_activation_tables

        has_act = any(
            isinstance(i, mybir.InstActivation)
            for blk in self.main_func.blocks
            for i in blk.instructions
        )
        if not has_act:
            return
        tabs = get_activation_tables(self.m.arch)
        tables = [
            (name, fns if name == "natural_log_exp_and_others" else set())
            for name, fns in tabs.items()
        ]
        _br.insert_act_table_loads(self, tables)

    nc.insert_act_table_loads = _act_table_loads_combined_set_only

    # x arrives pre-swizzled with hi/lo interleaved per chunk:
    #   x2[p, c, s, k, n] = (xh if s==0 else xl).T[k*128+p, c*CH+n]
    # so each chunk's per-partition DMA slice is one contiguous 8KB run.
    x2 = nc.declare_dram_parameter("x2", [128, nfull, 2, KS, CH], F16,
                                   isOutput=False)
    # last chunk as two 256-col halves, hi/lo (4KB runs):
    #   x2t[p, i, s, k, n] = (xh if s==0 else xl).T[k*128+p, nfull*CH+i*HW+n]
    x2t = nc.declare_dram_parameter("x2t", [128, 2, 2, KS, HW], F16,
                                    isOutput=False)
    # w1 halves pre-swizzled to [128, KS*H1]: one 800B contiguous run per
    # partition.
    w1h = nc.declare_dram_parameter("w1h", [128, KS * H1], F16, isOutput=False)
    w1l = nc.declare_dram_parameter("w1l", [128, KS * H1], F16, isOutput=False)
    w2t = nc.declare_dram_parameter("w2t", [H1, H2], F16, isOutput=False)
    w5t = nc.declare_dram_parameter("w5t", [H2, O], F16, isOutput=False)
    cs1 = nc.declare_dram_parameter("cs1", [H1, 2], F32, isOutput=False)
    cs2 = nc.declare_dram_parameter("cs2", [H2, 2], F32, isOutput=False)
    b5j = nc.declare_dram_parameter("b5j", [1, JB, O], F32, isOutput=False)
    # Output, stored untransposed, batch-on-partition:
    #   y[p, c, j, o] = out[c*512 + j*128 + p, o]
    y = nc.declare_dram_parameter("y", [128, nch, JB, O], F32, isOutput=True)

    with tile.TileContext(nc) as tc:
        from contextlib import ExitStack

        with ExitStack() as ctx:
            singles = ctx.enter_context(tc.tile_pool(name="singles", bufs=1))
            xpool = ctx.enter_context(tc.tile_pool(name="xpool", bufs=NBUF))
            tpool = ctx.enter_context(tc.tile_pool(name="tpool", bufs=1))
            mids = ctx.enter_context(tc.tile_pool(name="mids", bufs=3))
            # 8 PSUM banks exactly: ps1 x2, ps2 x2, ps5 x3 (ps5 is read by
            # exp in iteration c and by the subtract in iteration c+1, so
            # the fc5 of iteration c+2 needs a third bank to avoid a WAR
            # stall on the lagging DVE), fillers x1 (their WAW waits are
            # one group old -- consecutive groups are back-to-back anyway).
            p1pool = ctx.enter_context(tc.tile_pool(name="p1", bufs=2, space="PSUM"))
            p2pool = ctx.enter_context(tc.tile_pool(name="p2", bufs=2, space="PSUM"))
            p5pool = ctx.enter_context(tc.tile_pool(name="p5", bufs=3, space="PSUM"))
            pFpool = ctx.enter_context(tc.tile_pool(name="pF", bufs=1, space="PSUM"))

            # Identity first: the gpsimd engine builds it in ~1us so the
            # warmup fillers can start right after the preamble.
            ident = singles.tile([128, 128], F32)
            make_identity(nc, ident)
            identb = singles.tile([128, 128], F32)
            make_identity(nc, identb)

            xts = []

            def issue_x(c):
                # hi on the sync (SP) ring, lo on the scalar (ACT) ring.
                # Two rings are MANDATORY for wire rate: a single HWDGE
                # ring measured only ~320GB/s; two together reach ~400GB/s
                # (and gpsimd SWDGE measured ~25% slower -- not an option).
                # The cost of the scalar ring carrying lo: its DMA desc-gen
                # instructions carry semaphore-reuse waits that pace them
                # to the wire, and the exp/ln ACTIVATEs queued behind them
                # inherit a ~2-chunk lag; the stage_b split below makes
                # that lag harmless.
                xh_t = xpool.tile([128, KS, CH], F16, tag="xh", name="xh_t")
                nc.sync.dma_start(out=xh_t, in_=x2[:, c, 0])
                xl_t = xpool.tile([128, KS, CH], F16, tag="xl", name="xl_t")
                nc.scalar.dma_start(out=xl_t, in_=x2[:, c, 1])
                xts.append((xh_t, xl_t))

            xtt = []

            def issue_xt():
                for i in range(2):
                    th = tpool.tile([128, KS, HW], F16, tag=f"th{i}", name=f"th{i}")
                    nc.sync.dma_start(out=th, in_=x2t[:, i, 0])
                    tl = tpool.tile([128, KS, HW], F16, tag=f"tl{i}", name=f"tl{i}")
                    nc.scalar.dma_start(out=tl, in_=x2t[:, i, 1])
                    xtt.append((th, tl))

            # --- DMA preamble. fc1 weights lead each ring (~102KB, 0.55us;
            # fc1(0) cannot start without them). Consts are split across the
            # rings and slotted between chunk DMAs: w2/cs1 land before
            # stage_a(0) (~18us), b5j before stage_b(0), and the rings stay
            # byte-balanced so chunk completions never skew.
            w1h_sb = singles.tile([128, KS, H1], F16)
            nc.sync.dma_start(out=w1h_sb, in_=w1h[:, :])
            w1l_sb = singles.tile([128, KS, H1], F16)
            nc.scalar.dma_start(out=w1l_sb, in_=w1l[:, :])
            issue_x(0)
            w5_sb = singles.tile([H2, O], F16)
            nc.sync.dma_start(out=w5_sb, in_=w5t[:, :])
            cs2_sb = singles.tile([H2, 2], F32)
            nc.sync.dma_start(out=cs2_sb, in_=cs2[:, :])
            w2_sb = singles.tile([H1, H2], F16)
            nc.scalar.dma_start(out=w2_sb, in_=w2t[:, :])
            cs1_sb = singles.tile([H1, 2], F32)
            nc.scalar.dma_start(out=cs1_sb, in_=cs1[:, :])
            issue_x(1)
            b5j_sb = singles.tile([1, JB, O], F32)
            nc.sync.dma_start(out=b5j_sb, in_=b5j[:, :])
            ones_sb = singles.tile([1, 128], F32)
            nc.gpsimd.memset(ones_sb, 1.0)
            out_all = singles.tile([128, nch, JB, O], F32)
            for c in range(2, min(NBUF, nfull)):
                issue_x(c)

            def fillers(n, xt=None):
                # Keep-warm matmuls: pad PE idle so the HAM activity monitor
                # never re-throttles the clock. One accumulation group -> no
                # per-instruction WAW semaphore stalls; output never read.
                # Warmup form (xt=None): fp32 ident@ident, dependency-free
                # so it runs during the pre-chunk-0 window. Anchored form
                # (256-col, ~0.15us each): reads an already-landed x tile,
                # so it executes immediately when the PE reaches it --
                # emitted just before fc1(c), it fills the pre-chunk data
                # wait without delaying anything.
                if n <= 0:
                    return
                fp = pFpool.tile([128, CH], F32, tag="f", name="fill")
                if xt is None:
                    for i in range(n):
                        nc.tensor.matmul(fp[:, 0:128],
                                         lhsT=(ident if i % 2 == 0 else identb),
                                         rhs=ident,
                                         start=(i == 0), stop=(i == n - 1))
                else:
                    for i in range(n):
                        nc.tensor.matmul(fp[:, 0:256], lhsT=xt[:, 0, 0:128],
                                         rhs=xt[:, 0, 0:256],
                                         start=(i == 0), stop=(i == n - 1))

            def fc1(c):
                xh_t, xl_t = xts[c]
                ps1 = p1pool.tile([H1, CH], F32, tag="ps1", name="ps1")
                for k in range(KS):
                    nc.tensor.matmul(ps1, lhsT=w1h_sb[:, k, :], rhs=xh_t[:, k, :],
                                     start=(k == 0), stop=False)
                for k in range(KS):
                    nc.tensor.matmul(ps1, lhsT=w1l_sb[:, k, :], rhs=xl_t[:, k, :],
                                     start=False, stop=(k == KS - 1))
                return ps1

            def fc1_tail(i):
                th, tl = xtt[i]
                ps1 = p1pool.tile([H1, HW], F32, tag="ps1", name="ps1")
                for k in range(KS):
                    nc.tensor.matmul(ps1, lhsT=w1h_sb[:, k, :], rhs=th[:, k, :],
                                     start=(k == 0), stop=False)
                for k in range(KS):
                    nc.tensor.matmul(ps1, lhsT=w1l_sb[:, k, :], rhs=tl[:, k, :],
                                     start=False, stop=(k == KS - 1))
                return ps1

            def stage_a(ps1, w=CH):
                """sign1 -> fc2 -> sign2 for one chunk; returns (y1, y2).
                The signs run on the vector engine as {0,1} comparisons
                (weights/thresholds pre-folded on host) -- the ACT engine
                was the per-chunk straggler (lo descriptor-gen + exp/ln),
                and late sign1 stalled fc2 on the PE every chunk. GPSIMD
                cannot touch PSUM, so the DVE is the only other option."""
                y1 = mids.tile([H1, w], F16, tag="y1", name="y1")
                nc.vector.tensor_scalar(y1, ps1, cs1_sb[:, 0:1], None,
                                        OP.is_ge)
                ps2 = p2pool.tile([H2, w], F32, tag="ps2", name="ps2")
                nc.tensor.matmul(ps2, lhsT=w2_sb, rhs=y1, start=True, stop=True)
                y2 = mids.tile([H2, w], F16, tag="y2", name="y2", bufs=4)
                nc.vector.tensor_scalar(y2, ps2, cs2_sb[:, 0:1], None,
                                        OP.is_ge)
                return y1, y2

            def stage_b1(y2, jlo=0, njb=JB):
                """fc5 + exp for one chunk; returns (ps5, e). b5 (minus the
                sign colsum correction) rides INTO the fc5 PSUM accumulation
                as a rank-1 ones-x-b5row matmul opening the group (~0.1us
                PE) -- this removes the DVE multiply against exp(b5) and one
                cross-engine hop from the per-chunk softmax chain."""
                ps5 = p5pool.tile([128, njb, O], F32, tag="ps5", name="ps5")
                nc.tensor.matmul(ps5, lhsT=ones_sb[0:1, :],
                                 rhs=b5j_sb[0:1, jlo:jlo + njb, :],
                                 start=True, stop=False)
                for j in range(njb):
                    nc.tensor.matmul(ps5[:, j, :], lhsT=y2[:, j * 128:(j + 1) * 128],
                                     rhs=w5_sb, start=False, stop=True)
                e = mids.tile([128, njb, O], F32, tag="e", name="e")
                nc.scalar.activation(e, ps5, AF.Exp)
                return ps5, e

            def stage_b2(c, b1, jlo=0, njb=JB):
                """reduce + ln + subtract for one chunk; writes
                out_all[:, c, jlo:jlo+njb, :]. Runs ONE ITERATION LATER
                than stage_b1: the scalar engine's ACTIVATEs lag ~2 chunks
                behind readiness (queued behind wire-paced lo-DMA desc-gen),
                and if reduce/sub sat in the DVE queue the same iteration
                their chunk's exp was issued, the in-order DVE would stall
                on them, delaying the NEXT chunk's sign1 and stalling the
                PE (fc2 waits y1; fc1 waits the ps1 WAR). One iteration of
                separation absorbs the ACT lag. (|logits| <= 21 so no
                max-subtraction is needed for the softmax.)"""
                ps5, e = b1
                s = mids.tile([128, njb], F32, tag="s", name="s")
                nc.vector.reduce_sum(s, e, axis=AX.X)
                lse = mids.tile([128, njb], F32, tag="lse", name="lse")
                nc.scalar.activation(lse, s, AF.Ln)
                for j in range(njb):
                    nc.vector.tensor_scalar(
                        out_all[:, c, jlo + j, :], ps5[:, j, :],
                        lse[:, j:j + 1], None, OP.subtract)

            # Software pipeline, one chunk per iteration:
            #   [ramp fillers] fc1(c) | stage_a(c-1) | stage_b1(c-2) |
            #   stage_b2(c-3) | issue chunk c+NBUF. Every PE instruction's
            #   inputs are >=1 iteration old when the PE's in-order queue
            #   reaches it, so the PE only ever waits for the x DMAs -- and
            #   the fillers bridge those waits during the ramp.
            fillers(WARMUP_FILL)
            ps1s = {}
            y2s = {}
            b1s = {}
            for c in range(nfull):
                if c >= 1:
                    fillers(FILL_AT.get(c, 0), xt=xts[c - 1][0])
                ps1s[c] = fc1(c)
                if c >= 1:
                    _, y2s[c - 1] = stage_a(ps1s.pop(c - 1))
                if c >= 2:
                    b1s[c - 2] = stage_b1(y2s.pop(c - 2))
                if c >= 3:
                    stage_b2(c - 3, b1s.pop(c - 3))
                if c + NBUF < nfull:
                    issue_x(c + NBUF)
                elif c + NBUF == nfull:
                    issue_xt()

            # Drain: the two last-chunk halves pipeline through the chain;
            # chunk 14's and half-chunk a's softmax stages are slotted into
            # fc1(15b)'s PE window so their DVE/ACT chains overlap the last
            # fc1, and the final half-chunk chain is all that trails.
            ps1a = fc1_tail(0)
            _, y2s[nfull - 1] = stage_a(ps1s.pop(nfull - 1))
            b1s[nfull - 2] = stage_b1(y2s.pop(nfull - 2))
            stage_b2(nfull - 3, b1s.pop(nfull - 3))
            # bulk of the output ships now, overlapped with the tail
            # compute, on the by-then-idle sync ring
            nc.sync.dma_start(out=y[:, 0:nfull - 2], in_=out_all[:, 0:nfull - 2])
            _, y2a = stage_a(ps1a, w=HW)
            b1s[nfull - 1] = stage_b1(y2s.pop(nfull - 1))
            stage_b2(nfull - 2, b1s.pop(nfull - 2))
            ps1b = fc1_tail(1)
            b1a = stage_b1(y2a, jlo=0, njb=JB // 2)
            stage_b2(nfull - 1, b1s.pop(nfull - 1))
            nc.sync.dma_start(out=y[:, nfull - 2:nfull],
                              in_=out_all[:, nfull - 2:nfull])
            _, y2b = stage_a(ps1b, w=HW)
            stage_b2(nfull, b1a, jlo=0, njb=JB // 2)
            b1b = stage_b1(y2b, jlo=JB // 2, njb=JB // 2)
            stage_b2(nfull, b1b, jlo=JB // 2, njb=JB // 2)
            # Keep the PE busy through the cross-engine drain chain: a PE
            # idle here trips the HAM half-clock right as the end-of-kernel
            # cleanup (barriers + semaphore resets, ~5us) starts, doubling
            # it. Anchored on the last tail tile so they can't be hoisted.
            fillers(20, xt=xtt[1][0])
            nc.scalar.dma_start(out=y[:, nfull:], in_=out_all[:, nfull:])

    nc.finalize()
    return nc


def _prep_inputs(x, w1, b1, g1, be1, m1, v1, w2, b2, g2, be2, m2, v2, w5, b5,
                 bpc: int = BPC, n_cores: int = N_CORES):
    f64 = np.float64
    w1s = np.where(w1 >= 0, 1.0, -1.0).astype(np.float32)
    w2s = np.where(w2 >= 0, 1.0, -1.0).astype(np.float32)
    w5s = np.where(w5 >= 0, 1.0, -1.0).astype(np.float32)

    def wswz(a):  # [D, H1] -> [128, KS*H1]: a.T[k*128+p, m] -> out[p, k*H1+m]
        return np.ascontiguousarray(
            a.reshape(KS, 128, H1).transpose(1, 0, 2).reshape(128, KS * H1))

    w1h = wswz(np.ascontiguousarray(w1s.T).astype(np.float16))
    w1l = wswz((np.ascontiguousarray(w1s.T) / LO).astype(np.float16))
    # sign activations run on the vector engine as b = (preact >= thr) in
    # {0,1}; the +-1 semantics fold into DOUBLED next-layer weights plus a
    # row/col-sum constant absorbed into the next threshold / bias
    # (y = 2b - 1  =>  y @ W = b @ 2W - sum(W)). Requires BN scale > 0,
    # which holds here (g = 1, v > 0).
    w2t = (2.0 * np.ascontiguousarray(w2s.T)).astype(np.float16)  # [H1, H2]
    w5t = (2.0 * np.ascontiguousarray(w5s.T)).astype(np.float16)  # [H2, O]
    rs2 = w2s.astype(f64).sum(axis=1)                             # [H2]
    cs5 = w5s.astype(f64).sum(axis=1)                             # [O]

    b5c = (b5.astype(f64) - cs5)
    b5jr = np.tile(b5c.astype(np.float32), 4).reshape(1, 4, O)

    s1 = (g1.astype(f64) / np.sqrt(v1.astype(f64) + EPS))
    t1 = s1 * (b1.astype(f64) - m1.astype(f64)) + be1.astype(f64)
    thr1 = -t1 / s1
    cs1 = np.stack([thr1, 0 * thr1], axis=1).astype(np.float32)   # [H1, 2]
    s2 = (g2.astype(f64) / np.sqrt(v2.astype(f64) + EPS))
    t2 = s2 * (b2.astype(f64) - m2.astype(f64)) + be2.astype(f64)
    thr2 = rs2 - t2 / s2
    cs2 = np.stack([thr2, 0 * thr2], axis=1).astype(np.float32)   # [H2, 2]

    x = np.asarray(x, dtype=np.float32)
    xh = x.astype(np.float16)
    xl = ((x - xh.astype(np.float32)) * LO).astype(np.float16)

    nfull = bpc // CH - 1

    def swizzle(a):  # [nfull*CH, D] -> [128, nfull, KS, CH]
        return np.ascontiguousarray(
            a.T.reshape(KS, 128, nfull, CH).transpose(1, 2, 0, 3))

    def swizzle_t(a):  # [CH, D] -> [128, 2, KS, HW]
        return np.ascontiguousarray(
            a.T.reshape(KS, 128, 2, HW).transpose(1, 2, 0, 3))

    in_maps = []
    for c in range(n_cores):
        rs = slice(c * bpc, c * bpc + nfull * CH)
        rt = slice(c * bpc + nfull * CH, (c + 1) * bpc)
        # x2[p, c, s, k, n]; x2t[p, i, s, k, n]
        x2 = np.ascontiguousarray(
            np.stack([swizzle(xh[rs]), swizzle(xl[rs])], axis=2))
        x2tt = np.ascontiguousarray(
            np.stack([swizzle_t(xh[rt]), swizzle_t(xl[rt])], axis=2))
        in_maps.append({
            "x2": x2, "x2t": x2tt,
            "w1h": w1h, "w1l": w1l, "w2t": w2t, "w5t": w5t,
            "cs1": cs1, "cs2": cs2, "b5j": b5jr,
        })
    return in_maps


def _decode_output(y_dev: np.ndarray, bpc: int) -> np.ndarray:
    # y_dev [128, nch, 4, O]: y_dev[p, c, j, o] = out[c*512 + j*128 + p, o]
    return np.ascontiguousarray(
        y_dev.transpose(1, 2, 0, 3).reshape(bpc, O))


_CACHED = {}


def kernel(**inputs) -> np.ndarray:
    from concourse.bass_utils import run_bass_kernel_spmd

    in_maps = _prep_inputs(**inputs)
    if "nc" not in _CACHED:
        _CACHED["nc"] = build_bass()
    nc = _CACHED["nc"]
    res = run_bass_kernel_spmd(nc, in_maps, list(range(N_CORES)))
    out = np.empty((B, O), dtype=np.float32)
    for c in range(N_CORES):
        out[c * BPC:(c + 1) * BPC] = _decode_output(res.results[c]["y"], BPC)
    return out
